# revision 2
# baseline (speedup 1.0000x reference)
"""Bahdanau additive attention Trainium2 Bass kernel (v2: fp8 DoubleRow).

Reference (per batch b):
    U = key @ W_encoder.T                  # [S, A]
    V = q @ W_decoder.T                    # [A]
    score = tanh(U + V) @ v[0]             # [S]
    w = softmax(score)                     # [S]
    context = w @ key                      # [KD]

Sharding: data-parallel over batch across 8 NeuronCores (4 batches/core),
weights replicated.

Numerics (validated against the fixed-seed reference in numpy,
max-norm rel err ~1.05e-2 vs the 2e-2 gate):
  - U matmul in fp8 e4m3 with DoubleRow perf mode (2 k-tiles of 128
    contracted per instruction at 0.5 cycles/row = 2-4x fp32r rate).
    W_encoder^T is pre-scaled by 256 on the host before the e4m3 cast
    (its entries ~N(0, 1/1024) would land in the subnormal range);
    the 1/256 is folded into the tanh activation's scale.
  - score matmul (v . tanh) in fp32r, context matmul in bf16.
  - exp without max subtraction (|score| <= ~3 on this data, and is
    bounded by sum|v| ~ 26 in general -- well inside fp32 range).

Host-side prep inside kernel() (layout/cast only, no model math):
  - keyT fp8: key transposed to [k, s] chunk tiles [128, ktp=4, 2, 512]
    (partition = k%128 within pair-tile, middle dim = DoubleRow pair).
    This kills the per-chunk PE transpose matmuls and the DVE cast
    copies the v1 kernel spent ~9% of PE time + most of DVE on.
  - key native bf16 [128, t=4, kd] chunk tiles for the context matmul
    (halves HBM traffic vs fp32).
  - W_encoder^T fp8 pre-packed [128, ktp, 2, AD]; W_decoder^T, q^T, v
    re-laid out so no transposes are needed on device.
V = q @ W_decoder.T is still computed on device (prep phase, overlapped
with the first key chunk DMAs).

Per-core steady-state per s-chunk of 512:
  1. DMA keyT fp8 chunk (SWDGE queue) + key-native bf16 chunk (SP queue).
  2. U^T a-tiles [128, 512] = 4 DoubleRow matmuls each, PSUM accum.
  3. ACT: tanh(U^T/256 + V[a]) from PSUM (V as per-partition bias).
  4. score [1, 512] = v.T @ tanh-tiles accumulated over a-tiles (fp32r).
  5. ACT: e = exp(score), accum_out -> chunk sum.
  6. PE-transpose e-row into e-column tile [128, 4] (bf16 out).
  7. context PSUM [1, 1024] += e-col.T @ key-native-bf16, accumulated
     across chunks (deferred one chunk so PE never waits on ACT's exp).
Batch epilogue: Z = sum of chunk sums, context * (1/Z) on DVE, DMA out.
"""
import sys
sys.path.insert(0, "/opt/trn_rl_repo")

from contextlib import ExitStack

import numpy as np
import ml_dtypes

import concourse.bass as bass
import concourse.tile as tile
from concourse import bacc, masks, mybir

dt = mybir.dt
AF = mybir.ActivationFunctionType
PM = mybir.MatmulPerfMode

# Full problem shape
B, S, KD, QD, AD = 32, 2048, 1024, 1024, 1024
N_CORES = 8
BS = B // N_CORES          # batches per core
SC = 512                   # s-chunk (columns per U matmul)
NSC = S // SC              # s-chunks per batch
NKT2 = KD // 256           # k pair-tiles (DoubleRow: 2x128 contraction each)
NAT = AD // 128            # a-tiles
NQT = QD // 128            # q-tiles
NKH = KD // 512            # context free-dim chunks
WE_SCALE = 256.0           # host pre-scale on W_encoder^T before e4m3 cast

E4 = ml_dtypes.float8_e4m3
BF16 = ml_dtypes.bfloat16


def build_kernel(nc, dyn_reps=0):
    """Emit the per-core kernel into `nc` (a bacc.Bacc).

    dyn_reps>0 wraps the main loop in a hardware For_i loop (timing
    amplification for the differenced benchmark).
    """
    f32, f32r, bf16, fp8 = dt.float32, dt.float32r, dt.bfloat16, dt.float8e4

    keyt8_d = nc.dram_tensor("keyt8", [BS, NSC, 128, NKT2 * 2 * SC], fp8,
                             kind="ExternalInput").ap()
    knat_d = nc.dram_tensor("knatb", [BS, NSC, 128, 4 * KD], bf16,
                            kind="ExternalInput").ap()
    wet8_d = nc.dram_tensor("wet8", [128, NKT2 * 2 * AD], fp8,
                            kind="ExternalInput").ap()
    wdt_d = nc.dram_tensor("wdt", [128, NQT * AD], f32,
                           kind="ExternalInput").ap()
    qt_d = nc.dram_tensor("qt", [128, NQT * BS], f32,
                          kind="ExternalInput").ap()
    vc_d = nc.dram_tensor("vc", [128, NAT], f32,
                          kind="ExternalInput").ap()
    out_d = nc.dram_tensor("out", [BS, KD], f32, kind="ExternalOutput").ap()

    with tile.TileContext(nc) as tc, ExitStack() as ctx:
        const = ctx.enter_context(tc.tile_pool(name="const", bufs=1))

        ident_f = const.tile([128, 128], f32, name="ident_f")
        masks.make_identity(nc, ident_f[:])
        one_f = const.tile([1, 1], f32, name="one_f")
        nc.gpsimd.memset(one_f[:], 1.0)

        # Persistent weights
        wet8 = const.tile([128, NKT2 * 2 * AD], fp8, name="wet8")
        wet8v = wet8[:].rearrange("p (tp i a) -> p tp i a", i=2, a=AD)
        vcols = const.tile([128, NAT], f32r, name="vcols")
        vbias = const.tile([128, NAT * BS], f32, name="vbias")
        vbias3 = vbias[:].rearrange("p (m b) -> p m b", b=BS)

        # ---------------- pools ----------------
        kpool = ctx.enter_context(tc.tile_pool(name="knat", bufs=3))
        ktpool = ctx.enter_context(tc.tile_pool(name="keyt8", bufs=3))
        thpool = ctx.enter_context(tc.tile_pool(name="tanh", bufs=2))
        spool = ctx.enter_context(tc.tile_pool(name="small", bufs=2))
        pp_u = ctx.enter_context(tc.tile_pool(name="pp_u", bufs=2, space="PSUM"))
        pp_s = ctx.enter_context(tc.tile_pool(name="pp_s", bufs=1, space="PSUM"))
        pp_c = ctx.enter_context(tc.tile_pool(name="pp_c", bufs=1, space="PSUM"))

        # ---------------- weight prep (once per core) ----------------
        # Weight DMAs ride the ACT HWDGE queue (idle during prep); key
        # chunk DMAs for (b=0, c=0) go out in parallel on SP/SWDGE.
        nc.scalar.dma_start(wet8[:], wet8_d)
        with tc.tile_pool(name="wprep", bufs=1) as wprep, \
                tc.tile_pool(name="pp_v", bufs=1, space="PSUM") as pp_v:
            qt = wprep.tile([128, NQT * BS], f32r, name="qt")
            nc.gpsimd.dma_start(qt[:], qt_d)
            qt3 = qt[:].rearrange("p (t b) -> p t b", b=BS)
            vcf = wprep.tile([128, NAT], f32, name="vcf")
            nc.scalar.dma_start(vcf[:], vc_d)
            nc.vector.tensor_copy(vcols[:], vcf[:])
            wdt = wprep.tile([128, NQT * AD], f32r, name="wdt")
            nc.gpsimd.dma_start(wdt[:], wdt_d)
            wdt3 = wdt[:].rearrange("p (t a) -> p t a", a=AD)

            # V^T = (q @ Wd.T) as [BS, AD] (contract q on partitions)
            psv = pp_v.tile([BS, AD], f32, name="psv")
            for h in range(NKH):
                for t in range(NQT):
                    nc.tensor.matmul(
                        psv[:, h * 512:(h + 1) * 512],
                        qt3[:, t, :], wdt3[:, t, h * 512:(h + 1) * 512],
                        start=(t == 0), stop=(t == NQT - 1))
            vnat = wprep.tile([BS, AD], f32, name="vnat")
            nc.vector.tensor_copy(vnat[:], psv[:])
            # transpose V [BS, AD] -> vbias [128, m, BS]
            for m in range(NAT):
                psb = pp_s.tile([128, BS], f32, name=f"psb{m}", tag="pse")
                nc.tensor.matmul(psb[:], vnat[:, m * 128:(m + 1) * 128],
                                 ident_f[:BS, :BS], is_transpose=True)
                nc.vector.tensor_copy(vbias3[:, m, :], psb[:])

        # ---------------- main streaming loop ----------------
        def emit_body(rep):
            for b in range(BS):
                tagb = f"r{rep}b{b}"
                zparts = spool.tile([1, NSC], f32, name=f"zp{tagb}",
                                    tag="zparts")
                ctx_ps = [pp_c.tile([1, 512], f32, name=f"ctx{tagb}_{h}",
                                    tag=f"ctx{h}") for h in range(NKH)]

                def emit_tail(c, erow, knat3):
                    # 6. e-row -> e-columns [128, 4] (fp32 transpose-mode)
                    pse = pp_s.tile([128, 4], f32, name=f"pse{tagb}c{c}",
                                    tag="pse")
                    for sp in range(4):
                        nc.tensor.matmul(pse[:, sp:sp + 1],
                                         erow[:, sp * 128:(sp + 1) * 128],
                                         one_f[:], is_transpose=True)
                    ecol = spool.tile([128, 4], bf16, name=f"ec{tagb}c{c}",
                                      tag="ecol")
                    nc.vector.tensor_copy(ecol[:], pse[:])

                    # 7. context accumulation (contract over s, bf16)
                    for sp in range(4):
                        for h in range(NKH):
                            nc.tensor.matmul(
                                ctx_ps[h][:], ecol[:, sp:sp + 1],
                                knat3[:, sp, h * 512:(h + 1) * 512],
                                start=(c == 0 and sp == 0),
                                stop=(c == NSC - 1 and sp == 3))

                pending = None
                for c in range(NSC):
                    # 1. key chunk DMAs (two layouts, two queues)
                    knat = kpool.tile([128, 4 * KD], bf16,
                                      name=f"knat{tagb}c{c}", tag="knat")
                    knat3 = knat[:].rearrange("p (t k) -> p t k", k=KD)
                    nc.sync.dma_start(knat[:], knat_d[b, c])
                    kt8 = ktpool.tile([128, NKT2 * 2 * SC], fp8,
                                      name=f"kt8{tagb}c{c}", tag="kt8")
                    kt8v = kt8[:].rearrange("p (tp i s) -> p tp i s",
                                            i=2, s=SC)
                    nc.gpsimd.dma_start(kt8[:], keyt8_d[b, c])

                    # 2+3. U^T a-tiles (fp8 DoubleRow), tanh(U/256+V) on ACT
                    ths = []
                    for m in range(NAT):
                        psu = pp_u.tile([128, SC], f32,
                                        name=f"psu{tagb}c{c}m{m}", tag="psu")
                        for tp in range(NKT2):
                            nc.tensor.matmul(
                                psu[:],
                                wet8v[:, tp, :, m * 128:(m + 1) * 128],
                                kt8v[:, tp, :, :],
                                start=(tp == 0), stop=(tp == NKT2 - 1),
                                perf_mode=PM.DoubleRow)
                        th = thpool.tile([128, SC], f32r,
                                         name=f"th{tagb}c{c}m{m}",
                                         tag=f"th{m}")
                        nc.scalar.activation(th[:], psu[:], AF.Tanh,
                                             bias=vbias3[:, m, b:b + 1],
                                             scale=1.0 / WE_SCALE)
                        ths.append(th)

                    # 4. score row (fp32r)
                    pss = pp_s.tile([1, SC], f32, name=f"pss{tagb}c{c}",
                                    tag="pss")
                    for m in range(NAT):
                        nc.tensor.matmul(pss[:], vcols[:, m:m + 1], ths[m][:],
                                         start=(m == 0), stop=(m == NAT - 1))

                    # 5. e = exp(score); chunk sum via accum_out
                    erow = spool.tile([1, SC], f32, name=f"erow{tagb}c{c}",
                                      tag="erow")
                    nc.scalar.activation(erow[:], pss[:], AF.Exp,
                                         accum_out=zparts[:, c:c + 1])

                    # 6+7 for the PREVIOUS chunk (deferred so the PE never
                    # stalls at the e-column matmuls waiting for ACT's exp)
                    if pending is not None:
                        emit_tail(*pending)
                    pending = (c, erow, knat3)
                emit_tail(*pending)

                # batch epilogue: normalize and store
                z = spool.tile([1, 1], f32, name=f"z{tagb}", tag="z")
                nc.vector.reduce_sum(z[:], zparts[:], axis=mybir.AxisListType.X)
                rz = spool.tile([1, 1], f32, name=f"rz{tagb}", tag="rz")
                nc.vector.reciprocal(rz[:], z[:])
                cout = spool.tile([1, KD], f32, name=f"cout{tagb}", tag="cout")
                for h in range(NKH):
                    nc.vector.tensor_scalar_mul(cout[:, h * 512:(h + 1) * 512],
                                                ctx_ps[h][:], rz[:])
                nc.sync.dma_start(out_d[b:b + 1, :], cout[:])

        if dyn_reps:
            with tc.For_i(0, dyn_reps, 1):
                emit_body(0)
        else:
            emit_body(0)
    return nc


def prep_inputs(key, q, We, Wd, v):
    """Host-side layout/cast prep. Returns per-core in_maps."""
    key8 = key.astype(E4)          # [B, S, KD] fp8, contiguous cast
    keybf = key.astype(BF16)       # [B, S, KD] bf16

    wet8 = np.ascontiguousarray(
        (We.T * WE_SCALE).astype(E4)            # [KD, AD]
        .reshape(NKT2, 2, 128, AD).transpose(2, 0, 1, 3)
    ).reshape(128, NKT2 * 2 * AD)
    wdt = np.ascontiguousarray(
        Wd.T.reshape(NQT, 128, AD).transpose(1, 0, 2)
    ).reshape(128, NQT * AD).astype(np.float32)
    vc = np.ascontiguousarray(
        v[0].reshape(NAT, 128).T).astype(np.float32)

    in_maps = []
    for cidx in range(N_CORES):
        sl = slice(cidx * BS, (cidx + 1) * BS)
        keyt8 = np.ascontiguousarray(
            key8[sl].reshape(BS, NSC, SC, NKT2, 2, 128)
            .transpose(0, 1, 5, 3, 4, 2)
        ).reshape(BS, NSC, 128, NKT2 * 2 * SC)
        knatb = np.ascontiguousarray(
            keybf[sl].reshape(BS, NSC, 4, 128, KD).transpose(0, 1, 3, 2, 4)
        ).reshape(BS, NSC, 128, 4 * KD)
        qt = np.ascontiguousarray(
            q[sl].T.reshape(NQT, 128, BS).transpose(1, 0, 2)
        ).reshape(128, NQT * BS).astype(np.float32)
        in_maps.append({
            "keyt8": keyt8, "knatb": knatb, "wet8": wet8,
            "wdt": wdt, "qt": qt, "vc": vc,
        })
    return in_maps


_CACHE = {}


def _get_compiled(cfg):
    if cfg not in _CACHE:
        nc = bacc.Bacc("TRN2", target_bir_lowering=False, debug=False)
        build_kernel(nc, dyn_reps=cfg)
        nc.compile()
        _CACHE[cfg] = nc
    return _CACHE[cfg]


def kernel(**inputs):
    from concourse.bass_utils import run_bass_kernel_spmd

    key = np.asarray(inputs["key"], dtype=np.float32)
    q = np.asarray(inputs["q"], dtype=np.float32)
    we = np.asarray(inputs["W_encoder"], dtype=np.float32)
    wd = np.asarray(inputs["W_decoder"], dtype=np.float32)
    v = np.asarray(inputs["v"], dtype=np.float32)

    nc = _get_compiled(0)
    in_maps = prep_inputs(key, q, we, wd, v)
    res = run_bass_kernel_spmd(nc, in_maps, list(range(N_CORES))).results
    return np.concatenate([r["out"] for r in res], axis=0)


if __name__ == "__main__":
    pass
